# revision 48
# baseline (speedup 1.0000x reference)
"""Trainium2 Bass kernel for nn_Attn_30623116820602.

Low-rank-projected causal multi-head attention:
  q/k/v = (x @ A) @ B  (rank 192), RoPE on q,k, causal softmax attention,
  output projection.  x: [128, 256, 768] fp32.

Sharding: pure data-parallel over batch (16 items per core, 8 cores).
Feature-major layout on device (d_model on partitions); host pre-transposes
x per core and post-transposes the result.

v2: bf16 matmul operands everywhere (psum stays f32).  RoPE rotate-half is
done on-chip: DVE partition-shifted copies + a sign-folded sin table, with
the final add on GpSimd (otherwise idle).  The causal mask is accumulated
into the score PSUM by an identity-stationary matmul of a -1e4 triangular
constant, so exp needs no post-mask and the fully-masked quadrant is never
computed.  Softmax denominators come from an all-ones [128,128] stationary
matmul whose output is already broadcast across partitions; one DVE
reciprocal and the normalization is fused into the PSUM->SBUF move of the
attention output.
"""

import math
import sys

sys.path.insert(0, "/opt/trn_rl_repo")

import numpy as np
import ml_dtypes


def _to_bf16(a):
    return a.astype(ml_dtypes.bfloat16)


B, T, D = 128, 256, 768
H, HD = 6, 128
RANK = 192  # padded to 256 on host
N_CORES = 8
B_LOC = B // N_CORES  # 16
N_PAIRS = B_LOC // 2  # 8 (2 batch items per pipeline iteration)
SCALE = 1.0 / math.sqrt(HD)

_CACHE = {}


def build_program(n_pairs=N_PAIRS):
    import concourse.tile as tile
    from concourse import bacc, mybir
    from contextlib import ExitStack

    f32 = mybir.dt.float32
    bf16 = mybir.dt.bfloat16
    TOK = n_pairs * 512

    nc = bacc.Bacc("TRN2", target_bir_lowering=False, debug=False,
                   num_devices=N_CORES)

    def din(name, shape, dt=bf16):
        return nc.dram_tensor(name, shape, dt, kind="ExternalInput").ap()

    xT = din("xT", [6, 128, TOK])
    qA_l, kA_l, vA_l = (din(n, [6, 128, 192]) for n in ("qA_l", "kA_l", "vA_l"))
    qB_l, kB_l, vB_l = (din(n, [2, 128, 768]) for n in ("qB_l", "kB_l", "vB_l"))
    ow_l = din("ow_l", [6, 128, 768])
    cos2 = din("cos2", [128, 512])
    sin2 = din("sin2", [128, 512])
    tri_m = din("tri_m", [128, 256])     # 0 / -1e4 triangle pair
    eye_m = din("eye_m", [128, 128])     # identity
    ones_m = din("ones_m", [128, 128])   # all ones
    outT = nc.dram_tensor("outT", [6, 128, TOK], bf16,
                          kind="ExternalOutput").ap()

    with tile.TileContext(nc) as tc:
        with ExitStack() as ctx:
            wp = ctx.enter_context(tc.tile_pool(name="w", bufs=1))
            xp = ctx.enter_context(tc.tile_pool(name="xt", bufs=2))
            xrp = ctx.enter_context(tc.tile_pool(name="xr", bufs=2))
            mp = ctx.enter_context(tc.tile_pool(name="msb", bufs=9))
            qkp = ctx.enter_context(tc.tile_pool(name="qk", bufs=2))
            ep = ctx.enter_context(tc.tile_pool(name="eexp", bufs=6))
            rp = ctx.enter_context(tc.tile_pool(name="rec", bufs=3))
            aop = ctx.enter_context(tc.tile_pool(name="ao", bufs=2))
            fp = ctx.enter_context(tc.tile_pool(name="fout", bufs=2))
            ps = ctx.enter_context(tc.tile_pool(name="ps", bufs=4, space="PSUM"))
            psa = ctx.enter_context(tc.tile_pool(name="psa", bufs=2, space="PSUM"))
            psb = ctx.enter_context(tc.tile_pool(name="psb", bufs=2, space="PSUM"))

            # ---- resident weights / constants (all bf16) ----
            def wload(name, src, shape, perm=None):
                t = wp.tile(shape, bf16, tag=name, name=name)
                s = src if perm is None else src.rearrange(perm)
                nc.gpsimd.dma_start(t[:], s)
                return t

            qA_s = wload("qA", qA_l, [128, 6, 192], "k p m -> p k m")
            kA_s = wload("kA", kA_l, [128, 6, 192], "k p m -> p k m")
            vA_s = wload("vA", vA_l, [128, 6, 192], "k p m -> p k m")
            qB_s = wload("qB", qB_l, [128, 2, 768], "k p m -> p k m")
            kB_s = wload("kB", kB_l, [128, 2, 768], "k p m -> p k m")
            vB_s = wload("vB", vB_l, [128, 2, 768], "k p m -> p k m")
            ow_s = wload("ow", ow_l, [128, 6, 768], "k p m -> p k m")
            cos_s = wload("cos", cos2, [128, 512])
            sinsg_s = wload("sinsg", sin2, [128, 512])
            tri_s = wload("tri", tri_m, [128, 256])
            eye_s = wload("eye", eye_m, [128, 128])
            ones_s = wload("ones", ones_m, [128, 128])

            def emit_outproj(aosb_prev, pr_prev, mts, half=None):
                # half=None: both batch items (512 tok); half=b: one item
                w = 512 if half is None else 256
                c0 = 0 if half in (None, 0) else 256
                tokp = slice(pr_prev * 512 + c0, pr_prev * 512 + c0 + w)
                for mt in mts:
                    fps = ps.tile([128, 512], f32, tag="ps", name="fps")
                    for kt in range(6):
                        mov = (aosb_prev[:, :, kt, :] if half is None
                               else aosb_prev[:, half, kt, :])
                        nc.tensor.matmul(
                            fps[:, 0:w],
                            ow_s[:, kt, mt * 128:(mt + 1) * 128],
                            mov,
                            start=(kt == 0), stop=(kt == 5))
                    fout = fp.tile([128, 512], bf16, tag="fout", name="fout")
                    nc.vector.tensor_copy(fout[:, 0:w], fps[:, 0:w])
                    nc.sync.dma_start(outT[mt, :, tokp], fout[:, 0:w])

            def emit_proj1(xt, A_s, pname):
                mm = [psb.tile([128, 512], f32, tag="psb", name="p1"),
                      psb.tile([128, 512], f32, tag="psb", name="p1")]
                for mt in range(2):
                    for kt in range(6):
                        nc.tensor.matmul(
                            mm[mt][:],
                            A_s[:, kt, mt * 64:mt * 64 + 128],
                            xt[:, kt, :],
                            start=(kt == 0), stop=(kt == 5))
                xr = xrp.tile([128, 2, 512], bf16, tag=f"xr_{pname}",
                              name=f"xr_{pname}")
                nc.scalar.copy(xr[:, 0, :], mm[0][:])
                nc.scalar.copy(xr[:, 1, :], mm[1][:])
                return xr

            def emit_proj2_head(xr, B_s, sb, h):
                hs = slice(h * 512, (h + 1) * 512)
                p_main = psa.tile([128, 512], f32, tag="psa", name="pm")
                for kt in range(2):
                    nc.tensor.matmul(
                        p_main[:],
                        B_s[:, kt, h * 128:(h + 1) * 128],
                        xr[:, kt, :],
                        start=(kt == 0), stop=(kt == 1))
                msb = mp.tile([128, 512], bf16, tag="msb", name="msb")
                nc.scalar.copy(msb[:], p_main[:])
                # rotate-half: partition swap via DMA (latency hidden by the
                # one-iteration pipeline slack), then aligned DVE multiplies
                mrot = mp.tile([128, 512], bf16, tag="mrot", name="mrot")
                nc.sync.dma_start(mrot[0:64, :], msb[64:128, :])
                nc.sync.dma_start(mrot[64:128, :], msb[0:64, :])
                trot = mp.tile([128, 512], bf16, tag="trot", name="trot")
                nc.vector.tensor_tensor(
                    sb[:, hs], msb[:], cos_s[:], mybir.AluOpType.mult)
                nc.vector.tensor_tensor(
                    trot[:], mrot[:], sinsg_s[:], mybir.AluOpType.mult)
                nc.gpsimd.tensor_tensor(
                    sb[:, hs], sb[:, hs], trot[:], mybir.AluOpType.add)

            def emit_vproj(xrv):
                vsb = qkp.tile([128, 4, 768], bf16, tag="vsb", name="vsb")
                for mt in range(4):
                    for nch in range(2):
                        vp = psb.tile([128, 512], f32, tag="psb", name="vp")
                        for kt in range(2):
                            nc.tensor.matmul(
                                vp[:, 0:384],
                                xrv[:, kt, mt * 128:(mt + 1) * 128],
                                vB_s[:, kt, nch * 384:(nch + 1) * 384],
                                start=(kt == 0), stop=(kt == 1))
                        nc.scalar.copy(vsb[:, mt, nch * 384:(nch + 1) * 384],
                                       vp[:, 0:384])
                return vsb

            def emit_att_bg(qkv, aosb, b, g):
                # E layout per (b, h): [128 keys, 384]:
                #   cols 0:256   = key-tile 0 vs q 0:256
                #   cols 256:384 = key-tile 1 vs q 128:256
                qsb, ksb, vsb = qkv
                Eg = []
                for hh in range(2):
                    h = 2 * g + hh
                    qall = slice(h * 512 + b * 256, h * 512 + b * 256 + 256)
                    qhi = slice(h * 512 + b * 256 + 128,
                                h * 512 + b * 256 + 256)
                    k0 = slice(h * 512 + b * 256, h * 512 + b * 256 + 128)
                    k1 = slice(h * 512 + b * 256 + 128,
                               h * 512 + b * 256 + 256)
                    sp = ps.tile([128, 512], f32, tag="ps", name="sp")
                    nc.tensor.matmul(sp[:, 0:256], ksb[:, k0], qsb[:, qall],
                                     start=True, stop=False,
                                     skip_group_check=True)
                    nc.tensor.matmul(sp[:, 256:384], ksb[:, k1], qsb[:, qhi],
                                     start=False, stop=False,
                                     skip_group_check=True)
                    # one matmul adds -1e4 to both causal triangles
                    # (cols 0:128 and 256:384) via a strided psum AP
                    nc.tensor.matmul(
                        sp[:].rearrange("p (a b) -> p a b", a=4)[:, 0:3:2, :],
                        eye_s[:], tri_s[:],
                        start=False, stop=True, skip_group_check=True)
                    E = ep.tile([128, 384], bf16, tag="E", name="E")
                    nc.scalar.activation(E[:], sp[:, 0:384],
                                         mybir.ActivationFunctionType.Exp,
                                         scale=SCALE)
                    Eg.append(E)
                # denominators, broadcast across partitions by the all-ones
                # stationary.  Single-start accumulation groups.
                dbc = ps.tile([128, 512], f32, tag="ps", name="dbc")
                for hh in range(2):
                    E = Eg[hh]
                    c = hh * 256
                    nc.tensor.matmul(dbc[:, c:c + 256], ones_s[:], E[:, 0:256],
                                     start=(hh == 0), stop=False,
                                     skip_group_check=True)
                    nc.tensor.matmul(dbc[:, c + 128:c + 256], ones_s[:],
                                     E[:, 256:384], start=False,
                                     stop=(hh == 1), skip_group_check=True)
                rec = rp.tile([128, 512], f32, tag="rec", name="rec")
                nc.vector.reciprocal_approx_fast(rec[:], dbc[:])
                # attention @ v, fused normalize on the psum->sbuf move
                o2 = ps.tile([128, 512], f32, tag="ps", name="o2")
                for hh in range(2):
                    h = 2 * g + hh
                    E = Eg[hh]
                    c = hh * 256
                    v0 = vsb[:, b * 2, h * 128:(h + 1) * 128]
                    v1 = vsb[:, b * 2 + 1, h * 128:(h + 1) * 128]
                    nc.tensor.matmul(o2[:, c:c + 256], v0, E[:, 0:256],
                                     start=(hh == 0), stop=False,
                                     skip_group_check=True)
                    nc.tensor.matmul(o2[:, c + 128:c + 256], v1, E[:, 256:384],
                                     start=False, stop=(hh == 1),
                                     skip_group_check=True)
                nc.vector.tensor_tensor(
                    aosb[:, b, 2 * g:2 * g + 2, :],
                    o2[:].rearrange("p (h q) -> p h q", h=2),
                    rec[:].rearrange("p (h q) -> p h q", h=2),
                    mybir.AluOpType.mult)

            # software pipeline, finely zipped: attention chunks of pair N-1
            # are interleaved between projection chunks of pair N so every
            # engine's (in-order) stream always has ready work nearby.
            prev_qkv = None
            prev_ao = None
            for pr in range(n_pairs):
                have_att = prev_qkv is not None
                aosb = (aop.tile([128, 2, 6, 256], bf16, tag="aosb",
                                 name="aosb") if have_att else None)

                def att(i):
                    if not have_att:
                        return
                    b, g = divmod(i, 3)
                    if g == 1 and prev_ao is not None:
                        emit_outproj(prev_ao[0], prev_ao[1],
                                     range(3 * b, 3 * b + 3))
                    emit_att_bg(prev_qkv, aosb, b, g)

                if pr == 0:
                    tok = slice(0, 512)
                    xt = xp.tile([128, 6, 512], bf16, tag="xt", name="xt")
                    nc.sync.dma_start(
                        xt[:], xT[:, :, tok].rearrange("k p t -> p k t"))
                else:
                    xt = next_xt
                if pr + 1 < n_pairs:
                    ntok = slice((pr + 1) * 512, (pr + 2) * 512)
                    next_xt = xp.tile([128, 6, 512], bf16, tag="xt", name="xt")
                    nc.sync.dma_start(
                        next_xt[:], xT[:, :, ntok].rearrange("k p t -> p k t"))
                xr_q = emit_proj1(xt, qA_s, "q")
                att(0)
                xr_k = emit_proj1(xt, kA_s, "k")
                att(1)
                xr_v = emit_proj1(xt, vA_s, "v")
                att(2)
                qsb = qkp.tile([128, 3072], bf16, tag="qsb", name="qsb")
                ksb = qkp.tile([128, 3072], bf16, tag="ksb", name="ksb")
                for hp in range(3):
                    emit_proj2_head(xr_q, qB_s, qsb, 2 * hp)
                    emit_proj2_head(xr_k, kB_s, ksb, 2 * hp)
                    emit_proj2_head(xr_q, qB_s, qsb, 2 * hp + 1)
                    emit_proj2_head(xr_k, kB_s, ksb, 2 * hp + 1)
                    att(3 + hp)
                vsb = emit_vproj(xr_v)
                if have_att:
                    prev_ao = (aosb, pr - 1)
                prev_qkv = (qsb, ksb, vsb)

            # tail: attention for the last pair
            aosb = aop.tile([128, 2, 6, 256], bf16, tag="aosb", name="aosb")
            for b in range(2):
                if prev_ao is not None:
                    emit_outproj(prev_ao[0], prev_ao[1],
                                 range(3 * b, 3 * b + 3))
                for g in range(3):
                    emit_att_bg(prev_qkv, aosb, b, g)
            prev_ao = (aosb, n_pairs - 1)
            emit_outproj(prev_ao[0], prev_ao[1], range(6), half=0)
            emit_outproj(prev_ao[0], prev_ao[1], range(6), half=1)

    nc.compile()
    return nc


def _rope_tables():
    inv = 1.0 / (10000.0 ** (np.arange(0, HD, 2, dtype=np.float32) / HD))
    t = np.arange(T, dtype=np.float32)
    freqs = np.outer(t, inv)                      # [T, 64]
    emb = np.concatenate([freqs, freqs], axis=-1)  # [T, 128]
    return np.cos(emb).astype(np.float32), np.sin(emb).astype(np.float32)


def _prep_shared(qA, qB, kA, kB, vA, vB, o_w):
    """Host-side weight/constant layouts (shared by all cores)."""
    def a_layout(A):  # [768,192] -> [6,128,192]
        return _to_bf16(np.ascontiguousarray(A.reshape(6, 128, RANK)))

    def b_layout(Bm):  # [192,768] -> overlapped [2,128,768]
        Bp = np.zeros((2, 128, D), np.float32)
        Bp[0, 0:64] = Bm[0:64]
        Bp[1] = Bm[64:192]
        return _to_bf16(np.ascontiguousarray(Bp))

    cos, sin = _rope_tables()
    cosT = np.ascontiguousarray(cos.T)  # [128, 256]
    sinT = np.ascontiguousarray(sin.T)
    cos2 = np.concatenate([cosT, cosT], axis=1)  # [128, 512] (2 batch items)
    sinsg2 = np.concatenate([sinT, sinT], axis=1).copy()
    sinsg2[0:64] = -sinsg2[0:64]   # mrot[p<64] = msb[p+64] pairs with -sin

    # additive causal mask: the two -1e4 triangles (key-tile0 vs q 0:128,
    # key-tile1 vs q 128:256 -- identical patterns), stored adjacently
    p = np.arange(128)[:, None]
    c1 = np.arange(128)[None, :]
    tri1 = np.where(p > c1, -10000.0, 0.0).astype(np.float32)
    tri = np.concatenate([tri1, tri1], axis=1)  # [128, 256]

    return {
        "qA_l": a_layout(qA), "kA_l": a_layout(kA), "vA_l": a_layout(vA),
        "qB_l": b_layout(qB), "kB_l": b_layout(kB), "vB_l": b_layout(vB),
        "ow_l": _to_bf16(np.ascontiguousarray(o_w.reshape(6, 128, D))),
        "cos2": _to_bf16(cos2), "sin2": _to_bf16(sinsg2),
        "tri_m": _to_bf16(tri),
        "eye_m": _to_bf16(np.eye(128, dtype=np.float32)),
        "ones_m": _to_bf16(np.ones((128, 128), np.float32)),
    }


def x_to_xT(xc):
    """[b, T, D] -> [6, 128, b*T] feature-major, batch-major tokens."""
    nb = xc.shape[0]
    return _to_bf16(np.ascontiguousarray(
        xc.reshape(nb, T, 6, 128).transpose(2, 3, 0, 1).reshape(6, 128, nb * T)))


def outT_to_out(oT, nb):
    return np.ascontiguousarray(
        oT.astype(np.float32).reshape(6, 128, nb, T)
        .transpose(2, 3, 0, 1).reshape(nb, T, D))


def kernel(x, qA, qB, kA, kB, vA, vB, o_w):
    from concourse import bass_utils

    if "nc" not in _CACHE:
        _CACHE["nc"] = build_program(N_PAIRS)
    nc = _CACHE["nc"]

    shared = _prep_shared(
        np.asarray(qA, np.float32), np.asarray(qB, np.float32),
        np.asarray(kA, np.float32), np.asarray(kB, np.float32),
        np.asarray(vA, np.float32), np.asarray(vB, np.float32),
        np.asarray(o_w, np.float32))
    x = np.asarray(x, np.float32)

    in_maps = []
    for c in range(N_CORES):
        m = dict(shared)
        m["xT"] = x_to_xT(x[c * B_LOC:(c + 1) * B_LOC])
        in_maps.append(m)

    res = bass_utils.run_bass_kernel_spmd(
        nc, in_maps, core_ids=list(range(N_CORES)))
    out = np.empty((B, T, D), np.float32)
    for c in range(N_CORES):
        out[c * B_LOC:(c + 1) * B_LOC] = outT_to_out(
            res.results[c]["outT"], B_LOC)
    return out


# revision 49
# speedup vs baseline: 1.0462x; 1.0462x over previous
"""Trainium2 Bass kernel for nn_Attn_30623116820602.

Low-rank-projected causal multi-head attention:
  q/k/v = (x @ A) @ B  (rank 192), RoPE on q,k, causal softmax attention,
  output projection.  x: [128, 256, 768] fp32.

Sharding: pure data-parallel over batch (16 items per core, 8 cores).
Feature-major layout on device (d_model on partitions); host pre-transposes
x per core and post-transposes the result.

v2: bf16 matmul operands everywhere (psum stays f32).  RoPE rotate-half is
done on-chip: DVE partition-shifted copies + a sign-folded sin table, with
the final add on GpSimd (otherwise idle).  The causal mask is accumulated
into the score PSUM by an identity-stationary matmul of a -1e4 triangular
constant, so exp needs no post-mask and the fully-masked quadrant is never
computed.  Softmax denominators come from an all-ones [128,128] stationary
matmul whose output is already broadcast across partitions; one DVE
reciprocal and the normalization is fused into the PSUM->SBUF move of the
attention output.
"""

import math
import sys

sys.path.insert(0, "/opt/trn_rl_repo")

import numpy as np
import ml_dtypes


def _to_bf16(a):
    return a.astype(ml_dtypes.bfloat16)


B, T, D = 128, 256, 768
H, HD = 6, 128
RANK = 192  # padded to 256 on host
N_CORES = 8
B_LOC = B // N_CORES  # 16
N_PAIRS = B_LOC // 2  # 8 (2 batch items per pipeline iteration)
SCALE = 1.0 / math.sqrt(HD)

_CACHE = {}


def build_program(n_pairs=N_PAIRS):
    import concourse.tile as tile
    from concourse import bacc, mybir
    from contextlib import ExitStack

    f32 = mybir.dt.float32
    bf16 = mybir.dt.bfloat16
    TOK = n_pairs * 512

    nc = bacc.Bacc("TRN2", target_bir_lowering=False, debug=False,
                   num_devices=N_CORES)

    def din(name, shape, dt=bf16):
        return nc.dram_tensor(name, shape, dt, kind="ExternalInput").ap()

    xT = din("xT", [6, 128, TOK])
    qA_l, kA_l, vA_l = (din(n, [6, 128, 192]) for n in ("qA_l", "kA_l", "vA_l"))
    qB_l, kB_l, vB_l = (din(n, [2, 128, 768]) for n in ("qB_l", "kB_l", "vB_l"))
    ow_l = din("ow_l", [6, 128, 768])
    cos2 = din("cos2", [128, 512])
    sin2 = din("sin2", [128, 512])
    tri_m = din("tri_m", [128, 256])     # 0 / -1e4 triangle pair
    eye_m = din("eye_m", [128, 128])     # identity
    ones_m = din("ones_m", [128, 128])   # all ones
    outT = nc.dram_tensor("outT", [6, 128, TOK], bf16,
                          kind="ExternalOutput").ap()

    with tile.TileContext(nc) as tc:
        with ExitStack() as ctx:
            wp = ctx.enter_context(tc.tile_pool(name="w", bufs=1))
            xp = ctx.enter_context(tc.tile_pool(name="xt", bufs=2))
            xrp = ctx.enter_context(tc.tile_pool(name="xr", bufs=2))
            mp = ctx.enter_context(tc.tile_pool(name="msb", bufs=9))
            qkp = ctx.enter_context(tc.tile_pool(name="qk", bufs=2))
            ep = ctx.enter_context(tc.tile_pool(name="eexp", bufs=6))
            rp = ctx.enter_context(tc.tile_pool(name="rec", bufs=3))
            aop = ctx.enter_context(tc.tile_pool(name="ao", bufs=2))
            fp = ctx.enter_context(tc.tile_pool(name="fout", bufs=2))
            ps = ctx.enter_context(tc.tile_pool(name="ps", bufs=4, space="PSUM"))
            psa = ctx.enter_context(tc.tile_pool(name="psa", bufs=2, space="PSUM"))
            psb = ctx.enter_context(tc.tile_pool(name="psb", bufs=2, space="PSUM"))

            # ---- resident weights / constants (all bf16) ----
            def wload(name, src, shape, perm=None):
                t = wp.tile(shape, bf16, tag=name, name=name)
                s = src if perm is None else src.rearrange(perm)
                nc.gpsimd.dma_start(t[:], s)
                return t

            qA_s = wload("qA", qA_l, [128, 6, 192], "k p m -> p k m")
            kA_s = wload("kA", kA_l, [128, 6, 192], "k p m -> p k m")
            vA_s = wload("vA", vA_l, [128, 6, 192], "k p m -> p k m")
            qB_s = wload("qB", qB_l, [128, 2, 768], "k p m -> p k m")
            kB_s = wload("kB", kB_l, [128, 2, 768], "k p m -> p k m")
            vB_s = wload("vB", vB_l, [128, 2, 768], "k p m -> p k m")
            ow_s = wload("ow", ow_l, [128, 6, 768], "k p m -> p k m")
            cos_s = wload("cos", cos2, [128, 512])
            sinsg_s = wload("sinsg", sin2, [128, 512])
            tri_s = wload("tri", tri_m, [128, 256])
            eye_s = wload("eye", eye_m, [128, 128])
            ones_s = wload("ones", ones_m, [128, 128])

            def emit_outproj(aosb_prev, pr_prev, mts, half=None):
                # half=None: both batch items (512 tok); half=b: one item
                w = 512 if half is None else 256
                c0 = 0 if half in (None, 0) else 256
                tokp = slice(pr_prev * 512 + c0, pr_prev * 512 + c0 + w)
                for mt in mts:
                    fps = ps.tile([128, 512], f32, tag="ps", name="fps")
                    for kt in range(6):
                        mov = (aosb_prev[:, :, kt, :] if half is None
                               else aosb_prev[:, half, kt, :])
                        nc.tensor.matmul(
                            fps[:, 0:w],
                            ow_s[:, kt, mt * 128:(mt + 1) * 128],
                            mov,
                            start=(kt == 0), stop=(kt == 5))
                    fout = fp.tile([128, 512], bf16, tag="fout", name="fout")
                    nc.vector.tensor_copy(fout[:, 0:w], fps[:, 0:w])
                    nc.sync.dma_start(outT[mt, :, tokp], fout[:, 0:w])

            def emit_proj1(xt, A_s, pname):
                mm = [psb.tile([128, 512], f32, tag="psb", name="p1"),
                      psb.tile([128, 512], f32, tag="psb", name="p1")]
                for mt in range(2):
                    for kt in range(6):
                        nc.tensor.matmul(
                            mm[mt][:],
                            A_s[:, kt, mt * 64:mt * 64 + 128],
                            xt[:, kt, :],
                            start=(kt == 0), stop=(kt == 5))
                xr = xrp.tile([128, 2, 512], bf16, tag=f"xr_{pname}",
                              name=f"xr_{pname}")
                nc.scalar.copy(xr[:, 0, :], mm[0][:])
                nc.scalar.copy(xr[:, 1, :], mm[1][:])
                return xr

            def emit_proj2_head(xr, B_s, sb, h):
                hs = slice(h * 512, (h + 1) * 512)
                p_main = psa.tile([128, 512], f32, tag="psa", name="pm")
                for kt in range(2):
                    nc.tensor.matmul(
                        p_main[:],
                        B_s[:, kt, h * 128:(h + 1) * 128],
                        xr[:, kt, :],
                        start=(kt == 0), stop=(kt == 1))
                msb = mp.tile([128, 512], bf16, tag="msb", name="msb")
                nc.scalar.copy(msb[:], p_main[:])
                # rotate-half: partition swap via DMA (latency hidden by the
                # one-iteration pipeline slack), then aligned DVE multiplies
                mrot = mp.tile([128, 512], bf16, tag="mrot", name="mrot")
                nc.sync.dma_start(mrot[0:64, :], msb[64:128, :])
                nc.sync.dma_start(mrot[64:128, :], msb[0:64, :])
                trot = mp.tile([128, 512], bf16, tag="trot", name="trot")
                nc.vector.tensor_tensor(
                    sb[:, hs], msb[:], cos_s[:], mybir.AluOpType.mult)
                nc.vector.tensor_tensor(
                    trot[:], mrot[:], sinsg_s[:], mybir.AluOpType.mult)
                nc.gpsimd.tensor_tensor(
                    sb[:, hs], sb[:, hs], trot[:], mybir.AluOpType.add)

            def emit_vproj(xrv):
                vsb = qkp.tile([128, 4, 768], bf16, tag="vsb", name="vsb")
                for mt in range(4):
                    for nch in range(2):
                        vp = psb.tile([128, 512], f32, tag="psb", name="vp")
                        for kt in range(2):
                            nc.tensor.matmul(
                                vp[:, 0:384],
                                xrv[:, kt, mt * 128:(mt + 1) * 128],
                                vB_s[:, kt, nch * 384:(nch + 1) * 384],
                                start=(kt == 0), stop=(kt == 1))
                        nc.scalar.copy(vsb[:, mt, nch * 384:(nch + 1) * 384],
                                       vp[:, 0:384])
                return vsb

            def emit_att_bg(qkv, aosb, b, g):
                # E layout per (b, h): [128 keys, 384]:
                #   cols 0:256   = key-tile 0 vs q 0:256
                #   cols 256:384 = key-tile 1 vs q 128:256
                qsb, ksb, vsb = qkv
                Eg = []
                for hh in range(2):
                    h = 2 * g + hh
                    qall = slice(h * 512 + b * 256, h * 512 + b * 256 + 256)
                    qhi = slice(h * 512 + b * 256 + 128,
                                h * 512 + b * 256 + 256)
                    k0 = slice(h * 512 + b * 256, h * 512 + b * 256 + 128)
                    k1 = slice(h * 512 + b * 256 + 128,
                               h * 512 + b * 256 + 256)
                    sp = ps.tile([128, 512], f32, tag="ps", name="sp")
                    nc.tensor.matmul(sp[:, 0:256], ksb[:, k0], qsb[:, qall],
                                     start=True, stop=False,
                                     skip_group_check=True)
                    nc.tensor.matmul(sp[:, 256:384], ksb[:, k1], qsb[:, qhi],
                                     start=False, stop=False,
                                     skip_group_check=True)
                    # one matmul adds -1e4 to both causal triangles
                    # (cols 0:128 and 256:384) via a strided psum AP
                    nc.tensor.matmul(
                        sp[:].rearrange("p (a b) -> p a b", a=4)[:, 0:3:2, :],
                        eye_s[:], tri_s[:],
                        start=False, stop=True, skip_group_check=True)
                    E = ep.tile([128, 384], bf16, tag="E", name="E")
                    nc.scalar.activation(E[:], sp[:, 0:384],
                                         mybir.ActivationFunctionType.Exp,
                                         scale=SCALE)
                    Eg.append(E)
                # denominators, broadcast across partitions by the all-ones
                # stationary.  Single-start accumulation groups.
                dbc = ps.tile([128, 512], f32, tag="ps", name="dbc")
                for hh in range(2):
                    E = Eg[hh]
                    c = hh * 256
                    nc.tensor.matmul(dbc[:, c:c + 256], ones_s[:], E[:, 0:256],
                                     start=(hh == 0), stop=False,
                                     skip_group_check=True)
                    nc.tensor.matmul(dbc[:, c + 128:c + 256], ones_s[:],
                                     E[:, 256:384], start=False,
                                     stop=(hh == 1), skip_group_check=True)
                rec = rp.tile([128, 512], f32, tag="rec", name="rec")
                nc.vector.reciprocal_approx_fast(rec[:], dbc[:])
                # attention @ v, fused normalize on the psum->sbuf move
                o2 = ps.tile([128, 512], f32, tag="ps", name="o2")
                for hh in range(2):
                    h = 2 * g + hh
                    E = Eg[hh]
                    c = hh * 256
                    v0 = vsb[:, b * 2, h * 128:(h + 1) * 128]
                    v1 = vsb[:, b * 2 + 1, h * 128:(h + 1) * 128]
                    nc.tensor.matmul(o2[:, c:c + 256], v0, E[:, 0:256],
                                     start=(hh == 0), stop=False,
                                     skip_group_check=True)
                    nc.tensor.matmul(o2[:, c + 128:c + 256], v1, E[:, 256:384],
                                     start=False, stop=(hh == 1),
                                     skip_group_check=True)
                nc.vector.tensor_tensor(
                    aosb[:, b, 2 * g:2 * g + 2, :],
                    o2[:].rearrange("p (h q) -> p h q", h=2),
                    rec[:].rearrange("p (h q) -> p h q", h=2),
                    mybir.AluOpType.mult)

            # software pipeline, finely zipped: attention chunks of pair N-1
            # are interleaved between projection chunks of pair N so every
            # engine's (in-order) stream always has ready work nearby.
            prev_qkv = None
            prev_ao = None
            for pr in range(n_pairs):
                have_att = prev_qkv is not None
                aosb = (aop.tile([128, 2, 6, 256], bf16, tag="aosb",
                                 name="aosb") if have_att else None)

                def att(i):
                    if not have_att:
                        return
                    b, g = divmod(i, 3)
                    if g == 0 and prev_ao is not None:
                        emit_outproj(prev_ao[0], prev_ao[1],
                                     range(3 * b, 3 * b + 3))
                    emit_att_bg(prev_qkv, aosb, b, g)

                tok = slice(pr * 512, (pr + 1) * 512)
                xt = xp.tile([128, 6, 512], bf16, tag="xt", name="xt")
                nc.sync.dma_start(xt[:],
                                  xT[:, :, tok].rearrange("k p t -> p k t"))
                xr_q = emit_proj1(xt, qA_s, "q")
                att(0)
                xr_k = emit_proj1(xt, kA_s, "k")
                att(1)
                xr_v = emit_proj1(xt, vA_s, "v")
                att(2)
                qsb = qkp.tile([128, 3072], bf16, tag="qsb", name="qsb")
                ksb = qkp.tile([128, 3072], bf16, tag="ksb", name="ksb")
                for hp in range(3):
                    emit_proj2_head(xr_q, qB_s, qsb, 2 * hp)
                    emit_proj2_head(xr_k, kB_s, ksb, 2 * hp)
                    emit_proj2_head(xr_q, qB_s, qsb, 2 * hp + 1)
                    emit_proj2_head(xr_k, kB_s, ksb, 2 * hp + 1)
                    att(3 + hp)
                vsb = emit_vproj(xr_v)
                if have_att:
                    prev_ao = (aosb, pr - 1)
                prev_qkv = (qsb, ksb, vsb)

            # tail: attention for the last pair
            aosb = aop.tile([128, 2, 6, 256], bf16, tag="aosb", name="aosb")
            for b in range(2):
                if prev_ao is not None:
                    emit_outproj(prev_ao[0], prev_ao[1],
                                 range(3 * b, 3 * b + 3))
                for g in range(3):
                    emit_att_bg(prev_qkv, aosb, b, g)
            prev_ao = (aosb, n_pairs - 1)
            emit_outproj(prev_ao[0], prev_ao[1], range(6), half=0)
            emit_outproj(prev_ao[0], prev_ao[1], range(6), half=1)

    nc.compile()
    return nc


def _rope_tables():
    inv = 1.0 / (10000.0 ** (np.arange(0, HD, 2, dtype=np.float32) / HD))
    t = np.arange(T, dtype=np.float32)
    freqs = np.outer(t, inv)                      # [T, 64]
    emb = np.concatenate([freqs, freqs], axis=-1)  # [T, 128]
    return np.cos(emb).astype(np.float32), np.sin(emb).astype(np.float32)


def _prep_shared(qA, qB, kA, kB, vA, vB, o_w):
    """Host-side weight/constant layouts (shared by all cores)."""
    def a_layout(A):  # [768,192] -> [6,128,192]
        return _to_bf16(np.ascontiguousarray(A.reshape(6, 128, RANK)))

    def b_layout(Bm):  # [192,768] -> overlapped [2,128,768]
        Bp = np.zeros((2, 128, D), np.float32)
        Bp[0, 0:64] = Bm[0:64]
        Bp[1] = Bm[64:192]
        return _to_bf16(np.ascontiguousarray(Bp))

    cos, sin = _rope_tables()
    cosT = np.ascontiguousarray(cos.T)  # [128, 256]
    sinT = np.ascontiguousarray(sin.T)
    cos2 = np.concatenate([cosT, cosT], axis=1)  # [128, 512] (2 batch items)
    sinsg2 = np.concatenate([sinT, sinT], axis=1).copy()
    sinsg2[0:64] = -sinsg2[0:64]   # mrot[p<64] = msb[p+64] pairs with -sin

    # additive causal mask: the two -1e4 triangles (key-tile0 vs q 0:128,
    # key-tile1 vs q 128:256 -- identical patterns), stored adjacently
    p = np.arange(128)[:, None]
    c1 = np.arange(128)[None, :]
    tri1 = np.where(p > c1, -10000.0, 0.0).astype(np.float32)
    tri = np.concatenate([tri1, tri1], axis=1)  # [128, 256]

    return {
        "qA_l": a_layout(qA), "kA_l": a_layout(kA), "vA_l": a_layout(vA),
        "qB_l": b_layout(qB), "kB_l": b_layout(kB), "vB_l": b_layout(vB),
        "ow_l": _to_bf16(np.ascontiguousarray(o_w.reshape(6, 128, D))),
        "cos2": _to_bf16(cos2), "sin2": _to_bf16(sinsg2),
        "tri_m": _to_bf16(tri),
        "eye_m": _to_bf16(np.eye(128, dtype=np.float32)),
        "ones_m": _to_bf16(np.ones((128, 128), np.float32)),
    }


def x_to_xT(xc):
    """[b, T, D] -> [6, 128, b*T] feature-major, batch-major tokens."""
    nb = xc.shape[0]
    return _to_bf16(np.ascontiguousarray(
        xc.reshape(nb, T, 6, 128).transpose(2, 3, 0, 1).reshape(6, 128, nb * T)))


def outT_to_out(oT, nb):
    return np.ascontiguousarray(
        oT.astype(np.float32).reshape(6, 128, nb, T)
        .transpose(2, 3, 0, 1).reshape(nb, T, D))


def kernel(x, qA, qB, kA, kB, vA, vB, o_w):
    from concourse import bass_utils

    if "nc" not in _CACHE:
        _CACHE["nc"] = build_program(N_PAIRS)
    nc = _CACHE["nc"]

    shared = _prep_shared(
        np.asarray(qA, np.float32), np.asarray(qB, np.float32),
        np.asarray(kA, np.float32), np.asarray(kB, np.float32),
        np.asarray(vA, np.float32), np.asarray(vB, np.float32),
        np.asarray(o_w, np.float32))
    x = np.asarray(x, np.float32)

    in_maps = []
    for c in range(N_CORES):
        m = dict(shared)
        m["xT"] = x_to_xT(x[c * B_LOC:(c + 1) * B_LOC])
        in_maps.append(m)

    res = bass_utils.run_bass_kernel_spmd(
        nc, in_maps, core_ids=list(range(N_CORES)))
    out = np.empty((B, T, D), np.float32)
    for c in range(N_CORES):
        out[c * B_LOC:(c + 1) * B_LOC] = outT_to_out(
            res.results[c]["outT"], B_LOC)
    return out
